# revision 5
# baseline (speedup 1.0000x reference)
"""Trainium2 Bass kernel for nn_BeliefPropagationSingle (chain junction-tree BP).

Decomposition (validated against the axon-jax reference: 0 NaN/inf class
mismatches, ~1e-6 absmax rel err in the numpy model):

  colsum_i / rowsum_i of the ORIGINAL theta are the only dense reductions.
  Forward:   w_i = select(m_i>0, u_i + s_i*m_i, 0)
     u_i[k] = sum_r colsumP_i[Tf_i[r,k]]          (layered gather tables)
     s_{i+1} = sumu_i + 2048*s_i + 0*s_i          (0*s emulates the backend's
                                                   reduce inf->NaN artifact)
  Backward:  base_i = (rowsum_{i+1} + 2048*w_i) - w_i
     q_i[k] = sum_r baseP_i[Tb_i[r,k]]
     S_i = sumq_i + 2048*S_{i+1} + 0*S_{i+1}
     b_i = select(m'_i>0, q_i + S_{i+1}*m'_i, 0)
  Output:    out_i = (theta_i + w_{i-1}[:,None]) + b_i[None,:]

Sharding: core c owns cliques {2c, 2c+1} and gather steps {2c, 2c+1} of both
sweeps. One AllGather shares rowsum/u/sumu; a tiny second AllGather shares
sumq. Cross-core reads use partition-id dynamic DMA offsets into a
zero-padded copy of the AllGather result.

Layouts: B2 = [16, 128] tile holding vector element k at (k//128, k%128);
         L  = [128, 16] at (k%128, k//128); fm = [1, 2048] free-major.
"""
import numpy as np

N = 16
D = 2048
V = D + 1
P = 128
NB = 16          # B2 partition count
RMAX = 8
NCORES = 8
W1 = 4 * D + 8   # AG1 payload width (f32): rowsum x2, u x2, sumu x2, pad
U_OFF = 2 * D
SU_OFF = 4 * D

_COMPILED = None


# ----------------------------------------------------------------- host prep

def _build_layer_table(idx_a, idx_b):
    """T[r, k] = idx_a[j] of the r-th j with idx_b[j]==k; pad V-1 (zero slot)."""
    m = np.bincount(idx_b, minlength=D)
    R = int(m.max())
    assert R <= RMAX, f"layer count {R} > {RMAX}"
    T = np.full((RMAX, D), V - 1, dtype=np.int64)
    pos = np.zeros(D, dtype=np.int64)
    order = np.argsort(idx_b, kind="stable")
    for j in order:
        k = idx_b[j]
        T[pos[k], k] = idx_a[j]
        pos[k] += 1
    return T, m.astype(np.float32)


def _pack_idxs(T):
    """Wrap [RMAX, D] table for ap_gather: core j handles k in [256j,256j+256);
    seq_j[t] = T[t//256, 256j + t%256]; idxs[16j+p, s] = seq_j[16s+p]."""
    idxs = np.zeros((P, P), np.int16)
    for j in range(8):
        seq = np.empty(D, np.int64)
        for r in range(RMAX):
            seq[r * 256:(r + 1) * 256] = T[r, 256 * j:256 * (j + 1)]
        idxs[16 * j:16 * j + 16, :] = seq.reshape(P, 16).T
    return idxs


def _prep_inputs(theta, iaf, ibf, iab, ibb):
    fwd = [_build_layer_table(iaf[i], ibf[i]) for i in range(N - 1)]
    bwd = [_build_layer_table(iab[i], ibb[i]) for i in range(N - 1)]
    fwd_packed = [_pack_idxs(T) for T, _ in fwd]
    bwd_packed = [_pack_idxs(T) for T, _ in bwd]
    dummy_packed = _pack_idxs(np.full((RMAX, D), V - 1, dtype=np.int64))
    zeros_b2 = np.zeros((NB, P), np.float32)

    def mask_m(fam, i):
        if 0 <= i < N - 1:
            m = fam[i][1]
            return (m > 0).astype(np.uint8).reshape(NB, P), m.reshape(NB, P)
        return zeros_b2.astype(np.uint8), zeros_b2

    theta = np.asarray(theta, dtype=np.float32)
    in_maps = []
    for c in range(NCORES):
        fidx = np.stack([
            fwd_packed[2 * c],
            fwd_packed[2 * c + 1] if 2 * c + 1 < N - 1 else dummy_packed,
        ])
        bidx = np.stack([
            bwd_packed[2 * c],
            bwd_packed[2 * c + 1] if 2 * c + 1 < N - 1 else dummy_packed,
        ])
        wmask, wm = zip(*[mask_m(fwd, 2 * c - 1 + s) for s in range(3)])
        bmask, bm = zip(*[mask_m(bwd, 2 * c + l) for l in range(2)])
        in_maps.append({
            "theta_loc": np.ascontiguousarray(theta[2 * c:2 * c + 2]),
            "fidx": fidx,
            "bidx": bidx,
            "wmask": np.stack(wmask).astype(np.uint8),
            "wm": np.stack(wm).astype(np.float32),
            "bmask": np.stack(bmask).astype(np.uint8),
            "bm": np.stack(bm).astype(np.float32),
        })
    return in_maps


# ------------------------------------------------------------------- device

def _build_program():
    import concourse.bass as bass
    import concourse.bass_isa as bass_isa
    import concourse.mybir as mybir
    import concourse.tile as tile
    import concourse.bacc as bacc
    from concourse.masks import make_identity

    f32 = mybir.dt.float32
    i16 = mybir.dt.int16
    Add = mybir.AluOpType.add
    Sub = mybir.AluOpType.subtract
    Mult = mybir.AluOpType.mult
    X = mybir.AxisListType.X

    nc = bacc.Bacc("TRN2", target_bir_lowering=False, debug=False,
                   num_devices=NCORES)

    theta_loc = nc.dram_tensor("theta_loc", [2, D, D], f32, kind="ExternalInput")
    fidx_d = nc.dram_tensor("fidx", [2, P, P], i16, kind="ExternalInput")
    bidx_d = nc.dram_tensor("bidx", [2, P, P], i16, kind="ExternalInput")
    wmask_d = nc.dram_tensor("wmask", [3, NB, P], mybir.dt.uint8, kind="ExternalInput")
    wm_d = nc.dram_tensor("wm", [3, NB, P], f32, kind="ExternalInput")
    bmask_d = nc.dram_tensor("bmask", [2, NB, P], mybir.dt.uint8, kind="ExternalInput")
    bm_d = nc.dram_tensor("bm", [2, NB, P], f32, kind="ExternalInput")
    out_loc = nc.dram_tensor("out_loc", [2, D, D], f32, kind="ExternalOutput")

    NT = D // P  # 16 row-tiles per clique

    with tile.TileContext(nc) as tc:
        with (
            tc.tile_pool(name="big", bufs=4) as big,
            tc.tile_pool(name="mid", bufs=2) as mid,
            tc.tile_pool(name="small", bufs=1) as small,
            tc.tile_pool(name="ps", bufs=2, space="PSUM") as ps,
            tc.tile_pool(name="pcs", bufs=1, space="PSUM") as pcs,
            tc.tile_pool(name="dram", bufs=1, space="DRAM") as dram,
        ):
            # ---------- persistent constants ----------
            ones = small.tile([P, 1], f32)
            nc.vector.memset(ones[:], 1.0)
            ident16 = small.tile([16, 16], f32)
            make_identity(nc, ident16[:])
            ident128 = small.tile([P, P], f32)
            make_identity(nc, ident128[:])

            fidx_sb = small.tile([P, 2 * P], i16)
            bidx_sb = small.tile([P, 2 * P], i16)
            for l in range(2):
                nc.sync.dma_start(out=fidx_sb[:, P * l:P * (l + 1)], in_=fidx_d[l])
                nc.sync.dma_start(out=bidx_sb[:, P * l:P * (l + 1)], in_=bidx_d[l])
            wmask_sb = small.tile([NB, 3 * P], mybir.dt.uint8)
            wm_sb = small.tile([NB, 3 * P], f32)
            for s in range(3):
                nc.sync.dma_start(out=wmask_sb[:, P * s:P * (s + 1)], in_=wmask_d[s])
                nc.sync.dma_start(out=wm_sb[:, P * s:P * (s + 1)], in_=wm_d[s])
            bmask_sb = small.tile([NB, 2 * P], mybir.dt.uint8)
            bm_sb = small.tile([NB, 2 * P], f32)
            for l in range(2):
                nc.sync.dma_start(out=bmask_sb[:, P * l:P * (l + 1)], in_=bmask_d[l])
                nc.sync.dma_start(out=bm_sb[:, P * l:P * (l + 1)], in_=bm_d[l])

            ag1_in = dram.tile([1, W1], f32)
            ag1_out = dram.tile([NCORES, W1], f32)
            ag1_pad = dram.tile([NCORES + 2, W1], f32)
            ag2_in = dram.tile([1, 8], f32)
            ag2_out = dram.tile([NCORES, 8], f32)
            b_dram = dram.tile([2, D], f32)

            z2k = small.tile([1, D], f32)
            nc.vector.memset(z2k[:], 0.0)

            # ---------- phase A + forward gathers (per local clique) ----------
            u_sb = small.tile([NB, 2 * P], f32)      # u for my steps 2c, 2c+1
            rs_sb = small.tile([NB, 2 * P], f32)     # rowsum B2 for my cliques
            sums_sb = small.tile([NB, 4], f32)       # col0,1: sumu; col2,3: sumq

            for l in range(2):
                csp = pcs.tile([1, D], f32, name=f"csp{l}", tag="csp")
                rsL = small.tile([P, NT], f32, name=f"rsL{l}")
                for t in range(NT):
                    th = big.tile([P, D], f32, name=f"thA{l}_{t}", tag="thA")
                    nc.sync.dma_start(out=th[:], in_=theta_loc[l, P * t:P * (t + 1), :])
                    for j in range(4):
                        nc.tensor.matmul(
                            out=csp[0:1, 512 * j:512 * (j + 1)],
                            lhsT=ones[:], rhs=th[:, 512 * j:512 * (j + 1)],
                            start=(t == 0), stop=(t == NT - 1),
                        )
                    nc.vector.tensor_reduce(out=rsL[:, t:t + 1], in_=th[:], axis=X, op=Add)
                # colsum padded free-major [1, V]
                csP = mid.tile([1, V], f32, name=f"csP{l}", tag="vecP")
                nc.vector.memset(csP[:], 0.0)
                for j in range(4):
                    nc.vector.tensor_copy(csP[0:1, 512 * j:512 * (j + 1)],
                                          csp[0:1, 512 * j:512 * (j + 1)])
                # u gather pipeline
                srcg = mid.tile([P, V], f32, name=f"srcgu{l}", tag="srcg")
                nc.gpsimd.partition_broadcast(srcg[:], csP[:], channels=P)
                gout = mid.tile([P, D], f32, name=f"goutu{l}", tag="gout")
                nc.gpsimd.ap_gather(out_ap=gout[:], in_ap=srcg[:],
                                    idxs_ap=fidx_sb[:, P * l:P * (l + 1)],
                                    channels=P, num_elems=V, d=1, num_idxs=D)
                comp = mid.tile([NB, RMAX * P], f32, name=f"compu{l}", tag="comp")
                for h in range(2):
                    nc.sync.dma_start(
                        out=comp[h:NB:2, :],
                        in_=gout[0:P:16, :].rearrange("j (r q) -> j r q", r=RMAX)[:, :, P * h:P * (h + 1)],
                    )
                nc.vector.tensor_reduce(
                    out=u_sb[:, P * l:P * (l + 1)],
                    in_=comp[:].rearrange("p (r f) -> p f r", r=RMAX),
                    axis=X, op=Add,
                )
                # sumu
                upart = small.tile([NB, 1], f32, name=f"upart{l}")
                nc.vector.tensor_reduce(out=upart[:], in_=u_sb[:, P * l:P * (l + 1)],
                                        axis=X, op=Add)
                nc.gpsimd.partition_all_reduce(sums_sb[:, l:l + 1], upart[:],
                                               channels=NB,
                                               reduce_op=bass_isa.ReduceOp.add)
                # rowsum L -> B2 via PE transpose
                rsT = ps.tile([NB, P], f32, name=f"rsT{l}", tag="rsT")
                nc.tensor.transpose(out=rsT[:], in_=rsL[:], identity=ident128[:])
                nc.vector.tensor_copy(rs_sb[:, P * l:P * (l + 1)], rsT[:])

            # ---------- AG1 ----------
            for l in range(2):
                nc.sync.dma_start(
                    out=ag1_in[0:1, D * l:D * (l + 1)].rearrange("one (q f) -> one q f", q=NB),
                    in_=rs_sb[:, P * l:P * (l + 1)])
                nc.sync.dma_start(
                    out=ag1_in[0:1, U_OFF + D * l:U_OFF + D * (l + 1)].rearrange("one (q f) -> one q f", q=NB),
                    in_=u_sb[:, P * l:P * (l + 1)])
                nc.sync.dma_start(out=ag1_in[0:1, SU_OFF + l:SU_OFF + l + 1],
                                  in_=sums_sb[0:1, l:l + 1])
            nc.sync.dma_start(out=ag1_in[0:1, SU_OFF + 2:W1],
                              in_=z2k[0:1, 0:W1 - SU_OFF - 2])
            nc.gpsimd.collective_compute(
                "AllGather", mybir.AluOpType.bypass,
                replica_groups=[list(range(NCORES))],
                ins=[ag1_in[:]], outs=[ag1_out[:]],
            )
            # padded copy (row 0 and row 9 zero) for clamp-free dynamic reads
            for row in (0, NCORES + 1):
                for o in range(0, 4 * D, D):
                    nc.sync.dma_start(out=ag1_pad[row:row + 1, o:o + D], in_=z2k[:])
                nc.sync.dma_start(out=ag1_pad[row:row + 1, 4 * D:W1], in_=z2k[0:1, 0:8])
            nc.sync.dma_start(out=ag1_pad[1:NCORES + 1, :], in_=ag1_out[:])

            # ---------- forward scalar chain (redundant on every core) ----------
            su_row = small.tile([1, 2 * NCORES], f32)
            nc.sync.dma_start(
                out=su_row[0:1, :].rearrange("one (a b) -> one a b", a=NCORES),
                in_=ag1_out[:, SU_OFF:SU_OFF + 2])
            s_sc = small.tile([1, 18], f32)
            nc.vector.memset(s_sc[:], 0.0)
            ctmp = small.tile([1, 2], f32)
            for i in range(N - 1):
                nc.vector.tensor_scalar(out=ctmp[0:1, 0:1], in0=s_sc[0:1, i + 1:i + 2],
                                        scalar1=2048.0, scalar2=su_row[0:1, i:i + 1],
                                        op0=Mult, op1=Add)
                nc.vector.tensor_scalar(out=ctmp[0:1, 1:2], in0=s_sc[0:1, i + 1:i + 2],
                                        scalar1=0.0, scalar2=None, op0=Mult)
                nc.vector.tensor_tensor(out=s_sc[0:1, i + 2:i + 3], in0=ctmp[0:1, 0:1],
                                        in1=ctmp[0:1, 1:2], op=Add)
            s_rep = small.tile([NB, 18], f32)
            nc.gpsimd.partition_broadcast(s_rep[:], s_sc[:], channels=NB)

            # ---------- w slots (steps 2c-1, 2c, 2c+1) ----------
            pid_s = nc.sync.partition_id()
            pid_v = nc.vector.partition_id()
            u_left = small.tile([NB, P], f32)
            nc.sync.dma_start(
                out=u_left[:],
                in_=ag1_pad[bass.ds(pid_s, 1), U_OFF + D:U_OFF + 2 * D].rearrange(
                    "one (q f) -> one q f", q=NB))
            w_sb = small.tile([NB, 3 * P], f32)
            nc.vector.memset(w_sb[:], 0.0)
            wtmp = small.tile([NB, P], f32)
            wpre = small.tile([NB, P], f32)
            for s in range(3):
                u_src = u_left[:] if s == 0 else u_sb[:, P * (s - 1):P * s]
                s_dyn = s_rep[:, bass.ds(pid_v + pid_v + s, 1)]
                nc.vector.tensor_scalar(out=wtmp[:], in0=wm_sb[:, P * s:P * (s + 1)],
                                        scalar1=s_dyn, scalar2=None, op0=Mult)
                nc.vector.tensor_tensor(out=wpre[:], in0=u_src, in1=wtmp[:], op=Add)
                nc.vector.copy_predicated(out=w_sb[:, P * s:P * (s + 1)],
                                          mask=wmask_sb[:, P * s:P * (s + 1)],
                                          data=wpre[:])

            # ---------- backward bases + q gathers ----------
            rs_right = small.tile([NB, P], f32)
            nc.sync.dma_start(
                out=rs_right[:],
                in_=ag1_pad[bass.ds(pid_s + 2, 1), 0:D].rearrange(
                    "one (q f) -> one q f", q=NB))
            q_sb = small.tile([NB, 2 * P], f32)
            for l in range(2):
                rsrc = rs_sb[:, P:2 * P] if l == 0 else rs_right[:]
                wsl = w_sb[:, P * (l + 1):P * (l + 2)]
                bt1 = small.tile([NB, P], f32, name=f"bt1_{l}", tag="bt1")
                bt2 = small.tile([NB, P], f32, name=f"bt2_{l}", tag="bt2")
                base = small.tile([NB, P], f32, name=f"base_{l}", tag="base")
                nc.vector.tensor_scalar(out=bt1[:], in0=wsl, scalar1=2048.0,
                                        scalar2=None, op0=Mult)
                nc.vector.tensor_tensor(out=bt2[:], in0=rsrc, in1=bt1[:], op=Add)
                nc.vector.tensor_tensor(out=base[:], in0=bt2[:], in1=wsl, op=Sub)
                baseP = mid.tile([1, V], f32, name=f"baseP{l}", tag="vecP")
                nc.vector.memset(baseP[:], 0.0)
                nc.sync.dma_start(
                    out=baseP[0:1, 0:D].rearrange("one (q f) -> one q f", q=NB),
                    in_=base[:])
                srcb = mid.tile([P, V], f32, name=f"srcb{l}", tag="srcg")
                nc.gpsimd.partition_broadcast(srcb[:], baseP[:], channels=P)
                goutb = mid.tile([P, D], f32, name=f"goutb{l}", tag="gout")
                nc.gpsimd.ap_gather(out_ap=goutb[:], in_ap=srcb[:],
                                    idxs_ap=bidx_sb[:, P * l:P * (l + 1)],
                                    channels=P, num_elems=V, d=1, num_idxs=D)
                compb = mid.tile([NB, RMAX * P], f32, name=f"compb{l}", tag="comp")
                for h in range(2):
                    nc.sync.dma_start(
                        out=compb[h:NB:2, :],
                        in_=goutb[0:P:16, :].rearrange("j (r q) -> j r q", r=RMAX)[:, :, P * h:P * (h + 1)],
                    )
                nc.vector.tensor_reduce(
                    out=q_sb[:, P * l:P * (l + 1)],
                    in_=compb[:].rearrange("p (r f) -> p f r", r=RMAX),
                    axis=X, op=Add,
                )
                qpart = small.tile([NB, 1], f32, name=f"qpart{l}")
                nc.vector.tensor_reduce(out=qpart[:], in_=q_sb[:, P * l:P * (l + 1)],
                                        axis=X, op=Add)
                nc.gpsimd.partition_all_reduce(sums_sb[:, 2 + l:3 + l], qpart[:],
                                               channels=NB,
                                               reduce_op=bass_isa.ReduceOp.add)

            # ---------- AG2 (sumq) ----------
            z8 = small.tile([1, 8], f32)
            nc.vector.memset(z8[:], 0.0)
            nc.vector.tensor_copy(z8[0:1, 0:1], sums_sb[0:1, 2:3])
            nc.vector.tensor_copy(z8[0:1, 1:2], sums_sb[0:1, 3:4])
            nc.sync.dma_start(out=ag2_in[:], in_=z8[:])
            nc.gpsimd.collective_compute(
                "AllGather", mybir.AluOpType.bypass,
                replica_groups=[list(range(NCORES))],
                ins=[ag2_in[:]], outs=[ag2_out[:]],
            )
            sq_row = small.tile([1, 2 * NCORES], f32)
            nc.sync.dma_start(
                out=sq_row[0:1, :].rearrange("one (a b) -> one a b", a=NCORES),
                in_=ag2_out[:, 0:2])
            S_sc = small.tile([1, 18], f32)
            nc.vector.memset(S_sc[:], 0.0)
            ctmp2 = small.tile([1, 2], f32)
            for i in range(N - 2, -1, -1):
                nc.vector.tensor_scalar(out=ctmp2[0:1, 0:1], in0=S_sc[0:1, i + 2:i + 3],
                                        scalar1=2048.0, scalar2=sq_row[0:1, i:i + 1],
                                        op0=Mult, op1=Add)
                nc.vector.tensor_scalar(out=ctmp2[0:1, 1:2], in0=S_sc[0:1, i + 2:i + 3],
                                        scalar1=0.0, scalar2=None, op0=Mult)
                nc.vector.tensor_tensor(out=S_sc[0:1, i + 1:i + 2], in0=ctmp2[0:1, 0:1],
                                        in1=ctmp2[0:1, 1:2], op=Add)
            S_rep = small.tile([NB, 18], f32)
            nc.gpsimd.partition_broadcast(S_rep[:], S_sc[:], channels=NB)

            # ---------- b slots ----------
            b_sb = small.tile([NB, 2 * P], f32)
            nc.vector.memset(b_sb[:], 0.0)
            btmp = small.tile([NB, P], f32)
            bpre = small.tile([NB, P], f32)
            for l in range(2):
                S_dyn = S_rep[:, bass.ds(pid_v + pid_v + l + 2, 1)]
                nc.vector.tensor_scalar(out=btmp[:], in0=bm_sb[:, P * l:P * (l + 1)],
                                        scalar1=S_dyn, scalar2=None, op0=Mult)
                nc.vector.tensor_tensor(out=bpre[:], in0=q_sb[:, P * l:P * (l + 1)],
                                        in1=btmp[:], op=Add)
                nc.vector.copy_predicated(out=b_sb[:, P * l:P * (l + 1)],
                                          mask=bmask_sb[:, P * l:P * (l + 1)],
                                          data=bpre[:])

            # ---------- phase C ----------
            for l in range(2):
                # w_{clique-1} = w slot l; b_clique = b slot l
                wT = ps.tile([P, NB], f32, name=f"wT{l}", tag="wT")
                nc.tensor.transpose(out=wT[:], in_=w_sb[:, P * l:P * (l + 1)],
                                    identity=ident16[:])
                wL = small.tile([P, NB], f32, name=f"wL{l}")
                nc.vector.tensor_copy(wL[:], wT[:])
                bfm = small.tile([1, D], f32, name=f"bfm{l}", tag="bfm")
                nc.sync.dma_start(
                    out=bfm[0:1, :].rearrange("one (q f) -> one q f", q=NB),
                    in_=b_sb[:, P * l:P * (l + 1)])
                nc.sync.dma_start(out=b_dram[l:l + 1, :], in_=bfm[:])
                brep = mid.tile([P, D], f32, name=f"brep{l}", tag="brep")
                nc.sync.dma_start(out=brep[:], in_=b_dram[l:l + 1, :].to_broadcast([P, D]))
                for t in range(NT):
                    th = big.tile([P, D], f32, name=f"thC{l}_{t}", tag="thA")
                    nc.sync.dma_start(out=th[:], in_=theta_loc[l, P * t:P * (t + 1), :])
                    tt = big.tile([P, D], f32, name=f"ttC{l}_{t}", tag="ttC")
                    nc.gpsimd.tensor_scalar(out=tt[:], in0=th[:],
                                            scalar1=wL[:, t:t + 1], scalar2=None,
                                            op0=Add)
                    nc.vector.tensor_tensor(out=tt[:], in0=tt[:], in1=brep[:], op=Add)
                    nc.sync.dma_start(out=out_loc[l, P * t:P * (t + 1), :], in_=tt[:])

    nc.compile()
    return nc


def kernel(theta, idx_a_fwd, idx_b_fwd, idx_a_bwd, idx_b_bwd):
    global _COMPILED
    from concourse.bass_utils import run_bass_kernel_spmd

    if _COMPILED is None:
        _COMPILED = _build_program()
    nc = _COMPILED

    in_maps = _prep_inputs(theta, idx_a_fwd, idx_b_fwd, idx_a_bwd, idx_b_bwd)
    res = run_bass_kernel_spmd(nc, in_maps, core_ids=list(range(NCORES)))
    out = np.empty((N, D, D), np.float32)
    for c in range(NCORES):
        out[2 * c:2 * c + 2] = res.results[c]["out_loc"]
    return out
